# revision 1
# baseline (speedup 1.0000x reference)
"""Trainium2 Bass kernel for the iterated tiny-CNN problem.

Per step (16 steps): h -> relu(b2 + w2 . tanh(b1 + conv3x3(pad(h), w1)))
with circular (wrap) padding when n == W, else constant 0.5 padding.

Strategy (data-parallel over batch, 4 images per core on 8 cores):
  - Whole per-core state (4 images of 512x512 fp32) lives in SBUF for all
    steps; HBM traffic is load-once / store-once.
  - Each image is split into 5 row-blocks stored in one SBUF tensor
    [128 partitions x 5*514 cols]:
        partitions 0..125 : "primary" image rows (126 rows; last block 8)
        partition  126    : halo row below (first primary row of next block)
        partition  127    : halo row above (last primary row of prev block)
        (runt block: partition 8 is its halo row below)
        col slot 0        : wrap column (col 511), slots 1..512: cols 0..511,
        col slot 513      : wrap column (col 0)
  - conv3x3 runs on the TensorEngine as banded [128->126] matmuls: the 3
    vertical taps are diagonals of a tridiagonal weight matrix (corner
    entries pick up the halo partitions); the 3 horizontal taps are 3
    PSUM-accumulating matmuls with rhs shifted by -1/0/+1 columns.
    2 channels x 3 shifts = 6 matmuls per block per step.
  - tanh(+b1) on ScalarE reading PSUM; conv2 1x1 + bias + relu on VectorE.
  - Halo rows refresh once per step with 4 SBUF->SBUF DMAs per image.

kernel(**inputs) takes the full unsharded inputs and returns the full
output; sharding/compile/run/gather happen inside.
"""

import numpy as np

B_FULL = 32
H = 512
W = 512
N_CORES = 8
IMGS = B_FULL // N_CORES          # images per core
NT = 5                            # row-blocks (tiles) per image
TM = 126                          # primary rows per full tile
RUNT = H - 4 * TM                 # primary rows in last tile (8)
COLS = W + 2                      # per-tile columns incl. wrap cols
P = 128

_KERNEL_CACHE = {}


def _build_bands(w1):
    """Banded lhsT matrices [128, 6*128] fp32, layout [k, (c*3+dj)*128 + m].

    B[k, m] = w1[c, 0, di, dj] for k = m + di - 1 (di in 0..2), m in 0..125.
    k == -1 maps to partition 127 (halo-above slot).  k == 126 is the
    halo-below slot (arises naturally at m == 125, di == 2).
    """
    bands = np.zeros((128, 6 * 128), dtype=np.float32)
    for c in range(2):
        for dj in range(3):
            col0 = (c * 3 + dj) * 128
            for m in range(TM):
                for di in range(3):
                    k = m + di - 1
                    if k == -1:
                        k = 127
                    bands[k, col0 + m] = np.float32(w1[c, 0, di, dj])
    return bands


def _split_waits(nc, max_inline=1):
    """The walrus build here allows only one sync-wait per instruction;
    hoist extra waits into preceding same-engine NoOps (what raw bass's
    explicit wait_ge does)."""
    import concourse.mybir as mybir
    total = 0
    for fn in nc.m.functions:
        for blk in fn.blocks:
            insts = list(blk.instructions)
            new = []
            for ins in insts:
                si = ins.sync_info
                ow = list(si.on_wait) if si is not None else []
                if len(ow) > max_inline:
                    for w in ow[:-max_inline]:
                        nop = mybir.InstNoOp(
                            name=nc.get_next_instruction_name(),
                            engine=ins.engine,
                            ins=[], outs=[],
                            sync_info=mybir.SyncInfo(on_wait=[w],
                                                     on_update=[]),
                        )
                        new.append(nop)
                        total += 1
                    ins.sync_info = mybir.SyncInfo(
                        on_wait=ow[-max_inline:],
                        on_update=list(si.on_update))
                new.append(ins)
            blk.instructions = new
    return total


def _build_nc(steps, wrap, w1, b1, w2, b2, dt16=False):
    import concourse.bass as bass
    import concourse.mybir as mybir
    from concourse.tile import TileContext

    dt = mybir.dt
    DT = dt.bfloat16 if dt16 else dt.float32
    Alu = mybir.AluOpType
    Act = mybir.ActivationFunctionType

    w20 = float(w2[0, 0, 0, 0])
    w21 = float(w2[0, 1, 0, 0])
    b1f = [float(b1[0]), float(b1[1])]
    b2f = float(b2[0])
    # conv2: u = w20*y0 + w21*y1 + b2, computed as
    #   t = (y_a * ratio) + y_b ; u = t * sfin + b2    with |ratio| <= 1
    if abs(w21) >= abs(w20):
        a_idx, ratio, sfin = 0, (w20 / w21 if w21 else 0.0), w21
    else:
        a_idx, ratio, sfin = 1, w21 / w20, w20

    def rap(base, extra, dims):
        """Raw AP into `base` (an AP) at base.offset + extra with explicit
        [step, count] dims; dims[0] is the partition dim."""
        return bass.AP(base.tensor, base.offset + extra, dims)

    nc = bass.Bass()
    xs = nc.dram_tensor("xs", [IMGS, H, W], dt.float32, kind="ExternalInput")
    bands = nc.dram_tensor("bands", [128, 6 * 128], DT,
                           kind="ExternalInput")
    out = nc.dram_tensor("out", [IMGS, H, W], dt.float32,
                         kind="ExternalOutput")

    # rounds: pairs of adjacent blocks per image, image-interleaved so
    # consecutive rounds touch different images (deep pipeline).
    rounds = []
    for i in range(IMGS):
        for tpair in ((0, 1), (2, 3), (4,)):
            rounds.append((i, tpair))

    with TileContext(nc) as tc:
        with (
            tc.tile_pool(name="state", bufs=1) as state_pool,
            tc.tile_pool(name="const", bufs=1) as const_pool,
            tc.tile_pool(name="psum", bufs=2, space="PSUM") as psum_pool,
            tc.tile_pool(name="scratch", bufs=4) as scratch_pool,
        ):
            band_t = const_pool.tile([128, 6 * 128], DT, tag="bands")
            nc.sync.dma_start(band_t[:, :], bands[:, :])
            bias_t = []
            for c in range(2):
                bt = const_pool.tile([P, 1], dt.float32, tag=f"bias{c}",
                                     name=f"bias{c}")
                nc.vector.memset(bt[:, :], b1f[c])
                bias_t.append(bt)

            state = []
            for i in range(IMGS):
                st = state_pool.tile([P, NT * COLS], DT,
                                     tag=f"state{i}", name=f"state{i}")
                state.append(st)
            pitch = [st.ap[0][0] for st in state]

            def lhsT(c, dj):
                col0 = (c * 3 + dj) * 128
                return band_t[:, col0:col0 + TM]

            def prim_rows(t):
                return TM if t < 4 else RUNT

            # fp32 staging for the load and store paths: HWDGE DMAs run in
            # parallel queues but can't cast, and gpsimd casting DMAs
            # serialize ~1us each on the Pool engine.  Stage fp32 + DVE cast.
            stage = []
            for i in range(IMGS):
                sg = state_pool.tile([P, NT * W], dt.float32,
                                     tag=f"stage{i}", name=f"stage{i}")
                stage.append(sg)
            sp_pitch = [sg.ap[0][0] for sg in stage]

            # ---- initial load ----
            for i in range(IMGS):
                nc.gpsimd.memset(state[i][:, :], 0.0)
            for t in range(NT):
                for i in range(IMGS):
                    pr = prim_rows(t)
                    nc.sync.dma_start(
                        stage[i][0:pr, t * W: (t + 1) * W],
                        xs[i, t * TM: t * TM + pr, :],
                    )
            for t in range(NT):
                for i in range(IMGS):
                    pr = prim_rows(t)
                    nc.vector.tensor_copy(
                        state[i][0:pr, t * COLS + 1: t * COLS + 1 + W],
                        stage[i][0:pr, t * W: (t + 1) * W],
                    )

            def emit_wrap_cols_init(i):
                # slot0 <- slot512 (col 511), slot513 <- slot1 (col 0)
                if wrap:
                    for t in range(NT):
                        src = rap(state[i], t * COLS + 1,
                                  [[pitch[i], TM], [511, 2]])
                        dst = rap(state[i], t * COLS + 513,
                                  [[pitch[i], TM], [-513, 2]])
                        nc.vector.tensor_copy(dst, src)
                else:
                    for t in range(NT):
                        nc.vector.memset(
                            state[i][:, t * COLS: t * COLS + 1], 0.5)
                        nc.vector.memset(
                            state[i][:, t * COLS + 513: t * COLS + 514], 0.5)

            def emit_halo_rows(i):
                if wrap:
                    # p126 of t0..t3 <- p0 of t1..t4
                    nc.sync.dma_start(state[i][126:127, 0:4 * COLS],
                                      state[i][0:1, COLS:5 * COLS])
                    # p8 of t4 <- p0 of t0
                    nc.sync.dma_start(state[i][8:9, 4 * COLS:5 * COLS],
                                      state[i][0:1, 0:COLS])
                    # p127 of t1..t4 <- p125 of t0..t3
                    nc.sync.dma_start(state[i][127:128, COLS:5 * COLS],
                                      state[i][125:126, 0:4 * COLS])
                    # p127 of t0 <- p7 of t4
                    nc.sync.dma_start(state[i][127:128, 0:COLS],
                                      state[i][7:8, 4 * COLS:5 * COLS])
                else:
                    st = state[i]
                    nc.vector.memset(st[126:127, 0:4 * COLS], 0.5)
                    nc.vector.memset(st[8:9, 4 * COLS:5 * COLS], 0.5)
                    nc.vector.memset(st[127:128, 0:5 * COLS], 0.5)

            def emit_halo_rows_all():
                for i in range(IMGS):
                    emit_halo_rows(i)

            for i in range(IMGS):
                emit_wrap_cols_init(i)
            emit_halo_rows_all()

            # ---- steps ----
            for s in range(steps):
                for (i, tpair) in rounds:
                    ntile = len(tpair)
                    fd = ntile * W
                    st = state[i]
                    t0 = tpair[0]
                    pw = prim_rows(tpair[-1])  # partition rows of last tile

                    ps = []
                    for c in range(2):
                        pt = psum_pool.tile([P, 2, W], dt.float32,
                                            tag=f"ps{c}", name=f"ps{c}")
                        ps.append(pt)
                    for c in range(2):
                        for j, t in enumerate(tpair):
                            for dj in range(3):
                                rhs = st[0:P, t * COLS + dj: t * COLS + dj + W]
                                nc.tensor.matmul(
                                    ps[c][0:TM, j, :], lhsT(c, dj), rhs,
                                    start=(dj == 0), stop=(dj == 2),
                                )

                    ys = []
                    for c in range(2):
                        yt = scratch_pool.tile([P, 2 * W], DT,
                                               tag=f"y{c}", name=f"y{c}")
                        pp = ps[c].ap[0][0]
                        pin = rap(ps[c], 0, [[pp, TM], [1, fd]])
                        nc.scalar.activation(yt[0:TM, 0:fd], pin, Act.Tanh,
                                             bias=bias_t[c][0:TM, :],
                                             scale=1.0)
                        ys.append(yt)

                    tb = scratch_pool.tile([P, 2 * W], DT,
                                           tag="tb", name="tb")
                    nc.vector.scalar_tensor_tensor(
                        tb[0:TM, 0:fd], ys[a_idx][0:TM, 0:fd], ratio,
                        ys[1 - a_idx][0:TM, 0:fd], Alu.mult, Alu.add)
                    ub = scratch_pool.tile([P, 2 * W], DT,
                                           tag="ub", name="ub")
                    nc.vector.tensor_scalar(
                        ub[0:TM, 0:fd], tb[0:TM, 0:fd], sfin, b2f,
                        Alu.mult, Alu.add)

                    # final relu -> state primary cols (per-tile partition
                    # count: full tiles 126, runt tile 8 to spare its halo).
                    # Last step writes the fp32 staging buffer instead (no
                    # halos needed; feeds plain parallel store DMAs).
                    up = ub.ap[0][0]
                    last = (s == steps - 1)
                    if ntile == 2:
                        if last:
                            dstp = rap(stage[i], t0 * W,
                                       [[sp_pitch[i], TM], [W, 2], [1, W]])
                        else:
                            dstp = rap(st, t0 * COLS + 1,
                                       [[pitch[i], TM], [COLS, 2], [1, W]])
                        usrc = rap(ub, 0, [[up, TM], [W, 2], [1, W]])
                        nc.vector.tensor_scalar_max(dstp, usrc, 0.0)
                        if wrap and not last:
                            wsrc = rap(st, t0 * COLS + 1,
                                       [[pitch[i], TM], [COLS, 2], [511, 2]])
                            wdst = rap(st, t0 * COLS + 513,
                                       [[pitch[i], TM], [COLS, 2], [-513, 2]])
                            nc.vector.tensor_copy(wdst, wsrc)
                    else:
                        if last:
                            dstp = rap(stage[i], t0 * W,
                                       [[sp_pitch[i], pw], [1, W]])
                        else:
                            dstp = rap(st, t0 * COLS + 1,
                                       [[pitch[i], pw], [1, W]])
                        usrc = rap(ub, 0, [[up, pw], [1, W]])
                        nc.vector.tensor_scalar_max(dstp, usrc, 0.0)
                        if wrap and not last:
                            wsrc = rap(st, t0 * COLS + 1,
                                       [[pitch[i], pw], [511, 2]])
                            wdst = rap(st, t0 * COLS + 513,
                                       [[pitch[i], pw], [-513, 2]])
                            nc.vector.tensor_copy(wdst, wsrc)
                    # image i fully updated once its runt round is done:
                    # refresh its halo rows immediately so next step's
                    # first rounds aren't gated on the end of this step.
                    if tpair == (4,) and s < steps - 1:
                        emit_halo_rows(i)


            # ---- store ----
            for t in range(NT):
                for i in range(IMGS):
                    pr = prim_rows(t)
                    nc.sync.dma_start(
                        out[i, t * TM: t * TM + pr, :],
                        stage[i][0:pr, t * W: (t + 1) * W],
                    )
    _split_waits(nc)
    return nc


class _Runner:
    """Persistent jitted shard_map runner for a built Bass module
    (mirrors concourse.bass2jax.run_bass_via_pjrt, but reusable across
    calls and usable with device-resident inputs for timing)."""

    def __init__(self, nc):
        import jax
        import numpy as _np
        import concourse.mybir as mybir
        from jax.sharding import Mesh, PartitionSpec
        from jax.experimental.shard_map import shard_map
        from concourse import bass2jax

        bass2jax.install_neuronx_cc_hook()
        assert nc.dbg_addr is None

        partition_name = (nc.partition_id_tensor.name
                          if nc.partition_id_tensor else None)
        in_names, out_names, out_avals = [], [], []
        for alloc in nc.m.functions[0].allocations:
            if not isinstance(alloc, mybir.MemoryLocationSet):
                continue
            name = alloc.memorylocations[0].name
            if alloc.kind == "ExternalInput":
                if name != partition_name:
                    in_names.append(name)
            elif alloc.kind == "ExternalOutput":
                out_names.append(name)
                out_avals.append(jax.core.ShapedArray(
                    tuple(alloc.tensor_shape), mybir.dt.np(alloc.dtype)))
        self.in_names = in_names
        self.out_names = out_names
        self.out_avals = out_avals
        all_in_names = in_names + out_names
        if partition_name is not None:
            all_in_names = all_in_names + [partition_name]

        def _body(*args):
            operands = list(args)
            if partition_name is not None:
                operands.append(bass2jax.partition_id_tensor())
            outs = bass2jax._bass_exec_p.bind(
                *operands,
                out_avals=tuple(out_avals),
                in_names=tuple(all_in_names),
                out_names=tuple(out_names),
                lowering_input_output_aliases=(),
                sim_require_finite=True,
                sim_require_nnan=True,
                nc=nc,
            )
            return tuple(outs)

        devices = jax.devices()[:N_CORES]
        self.mesh = Mesh(_np.asarray(devices), ("core",))
        n_all = len(in_names) + len(out_names)
        self.fn = jax.jit(
            shard_map(_body, mesh=self.mesh,
                      in_specs=(PartitionSpec("core"),) * n_all,
                      out_specs=(PartitionSpec("core"),) * len(out_names),
                      check_rep=False),
            keep_unused=True,
        )

    def concat_inputs(self, in_maps):
        """Per-core in_maps -> global concat arrays (+ zero out bufs)."""
        arrs = []
        for name in self.in_names:
            arrs.append(np.concatenate(
                [np.asarray(m[name]) for m in in_maps], axis=0))
        for av in self.out_avals:
            arrs.append(np.zeros((N_CORES * av.shape[0],) + av.shape[1:],
                                 av.dtype))
        return arrs

    def __call__(self, *arrs):
        return self.fn(*arrs)

    def run(self, in_maps):
        out_arrs = self.fn(*self.concat_inputs(in_maps))
        res = []
        for c in range(N_CORES):
            res.append({
                name: np.asarray(out_arrs[i]).reshape(
                    (N_CORES,) + self.out_avals[i].shape)[c]
                for i, name in enumerate(self.out_names)})
        return res


def _get_runner(key, steps, wrap, w1, b1, w2, b2, dt16):
    if key not in _KERNEL_CACHE:
        nc = _build_nc(steps, wrap, w1, b1, w2, b2, dt16=dt16)
        _KERNEL_CACHE[key] = _Runner(nc)
    return _KERNEL_CACHE[key]


def _prep(x, w1, b1, w2, b2, steps, n, dt16=True):
    x = np.asarray(x)
    w1 = np.asarray(w1, dtype=np.float32)
    b1 = np.asarray(b1, dtype=np.float32)
    w2 = np.asarray(w2, dtype=np.float32)
    b2 = np.asarray(b2, dtype=np.float32)
    steps = int(steps)
    n = int(n)
    wrap = (n == W)
    xf = np.ascontiguousarray(x.reshape(B_FULL, H, W).astype(np.float32))
    bands = _build_bands(w1)
    if dt16:
        import ml_dtypes
        bands = bands.astype(ml_dtypes.bfloat16)
    key = (steps, wrap, dt16, w1.tobytes(), b1.tobytes(), w2.tobytes(),
           b2.tobytes())
    runner = _get_runner(key, steps, wrap, w1, b1, w2, b2, dt16)
    in_maps = [{"xs": xf[c * IMGS:(c + 1) * IMGS], "bands": bands}
               for c in range(N_CORES)]
    return runner, in_maps


def kernel(x, w1, b1, w2, b2, steps, n):
    in_dtype = np.asarray(x).dtype
    runner, in_maps = _prep(x, w1, b1, w2, b2, steps, n)
    res = runner.run(in_maps)
    full = np.concatenate([r["out"] for r in res], axis=0)
    full = full.reshape(B_FULL, 1, H, W)
    return full.astype(in_dtype, copy=False)



# revision 5
# speedup vs baseline: 1.5691x; 1.5691x over previous
"""Trainium2 Bass kernel for the iterated tiny-CNN problem.

Per step (16 steps): h -> relu(b2 + w2 . tanh(b1 + conv3x3(pad(h), w1)))
with circular (wrap) padding when n == W, else constant 0.5 padding.

Data-parallel over batch: 4 images per core on 8 cores. Per-core design:

  - State: per image [128 partitions x 4 blocks x 515 cols] in fp8e4m3.
    Block b holds image rows 128b..128b+127 (rows in partitions). Block
    col layout: col 0 = wrapL (img col 511), cols 1..512 = img cols
    0..511, col 513 = wrapR (img col 0), col 514 = dead padding.
  - conv3x3 on TensorE as fp8 DoubleRow (DR) matmuls, 2 taps per pass:
    per 64-row x 2-channel tile (tiles A/B per block, out partitions
    c*64+m), 2 main DR matmuls (rhs slot dim = stride-1 col shift,
    covering taps dj={0,1} and {2,pad}) + 2 K=1 edge DR matmuls that
    read the out-of-block halo row in place.
    Weights are pre-scaled x8 to keep fp8 bands in the normal range;
    undone by the activation scale.
  - tanh(+b1) on ScalarE: one instr per block [128, 1024] PSUM->SBUF,
    per-partition channel bias, fp8 output y.
  - conv2 1x1 as one DR matmul per block (slot dim stride 512 selects
    the tile-A/B column window of y; block-diagonal weights) into a
    dedicated u PSUM bank.
  - +b2, relu, fp8 cast in one DVE tensor_scalar (add,max) per block;
    wrap cols refreshed by tiny Pool copies. Halo rows are read in
    place from neighbor blocks by the K=1 edge matmuls; all mains of an
    image-step are emitted before its state writes (WAR ordering).

kernel(**inputs) takes the full unsharded inputs and returns the full
output; sharding/compile/run/gather happen inside.
"""

import numpy as np

B_FULL = 32
H = 512
W = 512
N_CORES = 8
IMGS = B_FULL // N_CORES          # images per core
NBLK = 4                          # 128-row blocks per image
COLS = 515                        # wrapL + 512 + wrapR + dead
HSW = 516                         # hs tile per-block window width
KSC = 8.0                         # band pre-scale (fp8 range), undone in ACT
P = 128

_KERNEL_CACHE = {}


def _build_consts(w1, w2):
    """fp8 band tensors.

    mains [128, 4, 2, 128]: slot (t,d) at index 2*t + d//2; L[p, i, m']
      with m' = c*64 + m, p = 64*t + m + di - 1 (in-block only),
      dj = d + i <= 2.
    edges [1, 2, 2, 2, 128]: K=1 in-place edge bands (t, w, i, m):
      window w=0 covers dj={0,1} (dj=i), w=1 covers dj=2 (i=1 unused).
      Tile A hits out m=0 (di=0), tile B out m=63 (di=2).
    comb [128, 2, 128]: slot i selects tile (y col window i*512);
      C[c*64+m, 0, m] = C[c*64+m, 1, 64+m] = w2[0,c].
    """
    import ml_dtypes
    FP8 = ml_dtypes.float8_e4m3fn
    mains = np.zeros((P, 4, 2, P), np.float32)
    edges = np.zeros((1, 2, 2, 2, P), np.float32)
    comb = np.zeros((P, 2, P), np.float32)
    for t in range(2):
        base = 64 * t
        for c in range(2):
            for m in range(64):
                for di in range(3):
                    p = base + m + di - 1
                    if p < 0 or p > 127:
                        continue
                    for d in (0, 2):
                        for i in range(2):
                            dj = d + i
                            if dj <= 2:
                                mains[p, 2 * t + d // 2, i, c * 64 + m] = (
                                    w1[c, 0, di, dj] * KSC)
    for t in range(2):
        di = 0 if t == 0 else 2
        m = 0 if t == 0 else 63
        for c in range(2):
            for i in range(2):
                edges[0, t, 0, i, c * 64 + m] = w1[c, 0, di, i] * KSC
            edges[0, t, 1, 0, c * 64 + m] = w1[c, 0, di, 2] * KSC
    for c in range(2):
        for m in range(64):
            comb[c * 64 + m, 0, m] = w2[0, c, 0, 0]
            comb[c * 64 + m, 1, 64 + m] = w2[0, c, 0, 0]
    return (mains.reshape(P, 1024).astype(FP8),
            edges.reshape(1, 1024).astype(FP8),
            comb.reshape(P, 256).astype(FP8))


def _split_waits(nc, max_inline=1):
    """The walrus build allows only one sync-wait per instruction; hoist
    extra waits into preceding same-engine NoOps."""
    import concourse.mybir as mybir
    total = 0
    for fn in nc.m.functions:
        for blk in fn.blocks:
            insts = list(blk.instructions)
            new = []
            for ins in insts:
                si = ins.sync_info
                ow = list(si.on_wait) if si is not None else []
                if len(ow) > max_inline:
                    for w in ow[:-max_inline]:
                        nop = mybir.InstNoOp(
                            name=nc.get_next_instruction_name(),
                            engine=ins.engine,
                            ins=[], outs=[],
                            sync_info=mybir.SyncInfo(on_wait=[w],
                                                     on_update=[]),
                        )
                        new.append(nop)
                        total += 1
                    ins.sync_info = mybir.SyncInfo(
                        on_wait=ow[-max_inline:],
                        on_update=list(si.on_update))
                new.append(ins)
            blk.instructions = new
    return total


def _build_nc(steps, wrap, w1, b1, w2, b2, dt16=True, imgs=IMGS):
    import concourse.bass as bass
    import concourse.mybir as mybir
    from concourse.tile import TileContext

    dt = mybir.dt
    Alu = mybir.AluOpType
    Act = mybir.ActivationFunctionType
    DRm = mybir.MatmulPerfMode.DoubleRow

    b1f = [float(b1[0]), float(b1[1])]
    b2f = float(b2[0])
    pad = 0.5  # constant-pad value (h units), used when wrap is False

    def rap(tile, extra, dims):
        return bass.AP(tile.tensor, tile.offset + extra, dims)

    nc = bass.Bass()
    xs = nc.dram_tensor("xs", [imgs, H, W], dt.float32, kind="ExternalInput")
    mains_d = nc.dram_tensor("mains", [P, 1024], dt.float8e4,
                             kind="ExternalInput")
    edges_d = nc.dram_tensor("edges", [1, 1024], dt.float8e4,
                             kind="ExternalInput")
    comb_d = nc.dram_tensor("comb", [P, 256], dt.float8e4,
                            kind="ExternalInput")
    out = nc.dram_tensor("out", [imgs, H, W], dt.float32,
                         kind="ExternalOutput")

    with TileContext(nc) as tc:
        with (
            tc.tile_pool(name="state", bufs=1) as state_pool,
            tc.tile_pool(name="const", bufs=1) as const_pool,
            tc.tile_pool(name="psum", bufs=3, space="PSUM") as psum_pool,
            tc.tile_pool(name="upsum", bufs=2, space="PSUM") as upsum_pool,
            tc.tile_pool(name="work", bufs=3) as work_pool,
            tc.tile_pool(name="stg", bufs=2) as stg_pool,
        ):
            mains_t = const_pool.tile([P, 1024], dt.float8e4, tag="mains")
            edges_t = const_pool.tile([1, 1024], dt.float8e4, tag="edges")
            comb_t = const_pool.tile([P, 256], dt.float8e4, tag="comb")
            nc.sync.dma_start(mains_t[:, :], mains_d[:, :])
            nc.sync.dma_start(edges_t[:, :], edges_d[:, :])
            nc.sync.dma_start(comb_t[:, :], comb_d[:, :])
            bias_t = const_pool.tile([P, 1], dt.float32, tag="bias")
            nc.vector.memset(bias_t[0:64, :], b1f[0])
            nc.vector.memset(bias_t[64:128, :], b1f[1])

            state = []
            for i in range(imgs):
                st = state_pool.tile([P, NBLK * COLS], dt.float8e4,
                                     tag=f"state{i}", name=f"state{i}")
                state.append(st)
            spitch = [st.ap[0][0] for st in state]
            # hs[t][i]: halo-row staging, tile t in {A,B}; slot k serves
            # block (k+1)%4 for A (src = row 127 of block k) and block
            # (k-1)%4 for B (src = row 0 of block k).
            hs = [[state_pool.tile([2, NBLK * HSW], dt.float8e4,
                                   tag=f"hs{t}_{i}", name=f"hs{t}_{i}")
                   for i in range(imgs)] for t in range(2)]
            hpitch = [[h.ap[0][0] for h in hs[t]] for t in range(2)]

            mp = mains_t.ap[0][0]
            ep = edges_t.ap[0][0]
            cp = comb_t.ap[0][0]

            def lhsT_main(t, d):
                return rap(mains_t, (2 * t + d // 2) * 256,
                           [[mp, P], [P, 2], [1, P]])

            def lhsT_edge(t):
                return rap(edges_t, t * 256, [[ep, 2], [P, 2], [1, P]])

            lhsT_comb = rap(comb_t, 0, [[cp, P], [P, 2], [1, P]])

            def rhs_main(i, b, d):
                return rap(state[i], b * COLS + d,
                           [[spitch[i], P], [1, 2], [1, 512]])

            def rhs_edge(i, b, t):
                k = (b - 1) % NBLK if t == 0 else (b + 1) % NBLK
                return rap(hs[t][i], k * HSW,
                           [[hpitch[t][i], 2], [1, 2], [1, 512]])

            # ---- init ----
            for i in range(imgs):
                if wrap:
                    # only the dead col 514 needs a defined (finite) value;
                    # wrap cols are written by emit_wrap_cols below
                    nc.gpsimd.memset(
                        rap(state[i], 514, [[spitch[i], P], [COLS, NBLK],
                                            [1, 1]]), 0.0)
                else:
                    nc.gpsimd.memset(state[i][:, :], pad)
                for t in range(2):
                    nc.gpsimd.memset(hs[t][i][:, :],
                                     pad if not wrap else 0.0)
            for i in range(imgs):
                sg = stg_pool.tile([P, NBLK * 512], dt.float32, tag="stg",
                                   name=f"ld{i}")
                sp = sg.ap[0][0]
                # (p, b, c) <- x[i, 128b+p, c]; split in halves so casts
                # start while the second half is still in flight
                for h in range(2):
                    nc.sync.dma_start(
                        rap(sg, 2 * h * 512, [[sp, P], [512, 2], [1, 512]]),
                        bass.AP(xs, i * H * W + 2 * h * 65536,
                                [[512, P], [65536, 2], [1, 512]]))
                    for b in (2 * h, 2 * h + 1):
                        nc.vector.tensor_copy(
                            state[i][:, b * COLS + 1: b * COLS + 513],
                            sg[:, b * 512: (b + 1) * 512])

            def emit_wrap_cols(i, blocks=range(NBLK)):
                # col 0 <- col 512 (img col 511); col 513 <- col 1
                blocks = list(blocks)
                b0 = blocks[0]
                nbl = len(blocks)
                src = rap(state[i], b0 * COLS + 512,
                          [[spitch[i], P], [COLS, nbl], [-511, 2]])
                dst = rap(state[i], b0 * COLS,
                          [[spitch[i], P], [COLS, nbl], [513, 2]])
                nc.gpsimd.tensor_copy(dst, src)

            def emit_hs(i):
                # hsA[p, k*HSW + j] = state[127, k*COLS + 2p + j]
                # hsB[p, k*HSW + j] = state[0,  k*COLS + 2p + j]
                for t, row in ((0, 127), (1, 0)):
                    for pw in range(2):
                        src = rap(state[i], row * spitch[i] + 2 * pw,
                                  [[spitch[i], 1], [COLS, NBLK], [1, 513]])
                        dst = rap(hs[t][i], pw * hpitch[t][i],
                                  [[hpitch[t][i], 1], [HSW, NBLK],
                                   [1, 513]])
                        nc.sync.dma_start(dst, src)

            if wrap:
                for i in range(imgs):
                    emit_wrap_cols(i)
                    emit_hs(i)

            # ---- steps ----
            zstride = 512  # fp32 elems per psum bank
            for s in range(steps):
                last = s == steps - 1
                for i in range(imgs):
                    if last:
                        sg = stg_pool.tile([P, NBLK * 512], dt.float32,
                                           tag="stg", name=f"st{s}_{i}")
                        sp = sg.ap[0][0]
                    for g in range(2):
                        z = psum_pool.tile([P, 4, 512], dt.float32,
                                           tag="z", name=f"z{s}_{i}_{g}")
                        zp = z.ap[0][0]
                        y = work_pool.tile([P, 2048], dt.float8e4,
                                           tag="y", name=f"y{s}_{i}_{g}")
                        for b01 in range(2):
                            b = 2 * g + b01
                            for t in range(2):
                                beta = 2 * b01 + t
                                zo = z[:, beta, :]
                                nc.tensor.matmul(
                                    zo, lhsT_main(t, 0), rhs_main(i, b, 0),
                                    start=True, stop=False, perf_mode=DRm)
                                nc.tensor.matmul(
                                    zo, lhsT_main(t, 2), rhs_main(i, b, 2),
                                    start=False, stop=False, perf_mode=DRm)
                                nc.tensor.matmul(
                                    zo, lhsT_edge(t), rhs_edge(i, b, t),
                                    start=False, stop=True, perf_mode=DRm,
                                    tile_position=(0, 0))
                        nc.scalar.activation(
                            y[:, :], rap(z, 0, [[zp, P], [1, 2048]]),
                            Act.Tanh, bias=bias_t[:, :], scale=1.0 / KSC)
                        yp = y.ap[0][0]
                        for b01 in range(2):
                            rhs_y = rap(y, b01 * 1024,
                                        [[yp, P], [512, 2], [1, 512]])
                            nc.tensor.matmul(
                                z[:, 2 * b01, :], lhsT_comb, rhs_y,
                                start=True, stop=True, perf_mode=DRm)
                        usrc = rap(z, 0, [[zp, P], [2 * zstride, 2],
                                          [1, 512]])
                        if last:
                            dst = rap(sg, 2 * g * 512,
                                      [[sp, P], [512, 2], [1, 512]])
                        else:
                            dst = rap(state[i], 2 * g * COLS + 1,
                                      [[spitch[i], P], [COLS, 2], [1, 512]])
                        nc.vector.tensor_scalar(dst, usrc, b2f, 0.0,
                                                Alu.add, Alu.max)
                        if not last and wrap:
                            emit_wrap_cols(i, [2 * g, 2 * g + 1])
                    if last:
                        for h in range(2):
                            nc.sync.dma_start(
                                bass.AP(out, i * H * W + 2 * h * 65536,
                                        [[512, P], [65536, 2], [1, 512]]),
                                rap(sg, 2 * h * 512,
                                    [[sp, P], [512, 2], [1, 512]]))
                    elif wrap:
                        emit_hs(i)
    _split_waits(nc)
    return nc


class _Runner:
    """Persistent jitted shard_map runner for a built Bass module."""

    def __init__(self, nc):
        import jax
        import numpy as _np
        import concourse.mybir as mybir
        from jax.sharding import Mesh, PartitionSpec
        from jax.experimental.shard_map import shard_map
        from concourse import bass2jax

        bass2jax.install_neuronx_cc_hook()
        assert nc.dbg_addr is None

        partition_name = (nc.partition_id_tensor.name
                          if nc.partition_id_tensor else None)
        in_names, out_names, out_avals = [], [], []
        for alloc in nc.m.functions[0].allocations:
            if not isinstance(alloc, mybir.MemoryLocationSet):
                continue
            name = alloc.memorylocations[0].name
            if alloc.kind == "ExternalInput":
                if name != partition_name:
                    in_names.append(name)
            elif alloc.kind == "ExternalOutput":
                out_names.append(name)
                out_avals.append(jax.core.ShapedArray(
                    tuple(alloc.tensor_shape), mybir.dt.np(alloc.dtype)))
        self.in_names = in_names
        self.out_names = out_names
        self.out_avals = out_avals
        all_in_names = in_names + out_names
        if partition_name is not None:
            all_in_names = all_in_names + [partition_name]

        def _body(*args):
            operands = list(args)
            if partition_name is not None:
                operands.append(bass2jax.partition_id_tensor())
            outs = bass2jax._bass_exec_p.bind(
                *operands,
                out_avals=tuple(out_avals),
                in_names=tuple(all_in_names),
                out_names=tuple(out_names),
                lowering_input_output_aliases=(),
                sim_require_finite=True,
                sim_require_nnan=True,
                nc=nc,
            )
            return tuple(outs)

        devices = jax.devices()[:N_CORES]
        self.mesh = Mesh(_np.asarray(devices), ("core",))
        n_all = len(in_names) + len(out_names)
        self.fn = jax.jit(
            shard_map(_body, mesh=self.mesh,
                      in_specs=(PartitionSpec("core"),) * n_all,
                      out_specs=(PartitionSpec("core"),) * len(out_names),
                      check_rep=False),
            keep_unused=True,
        )

    def concat_inputs(self, in_maps):
        arrs = []
        for name in self.in_names:
            arrs.append(np.concatenate(
                [np.asarray(m[name]) for m in in_maps], axis=0))
        for av in self.out_avals:
            arrs.append(np.zeros((N_CORES * av.shape[0],) + av.shape[1:],
                                 av.dtype))
        return arrs

    def __call__(self, *arrs):
        return self.fn(*arrs)

    def run(self, in_maps):
        out_arrs = self.fn(*self.concat_inputs(in_maps))
        res = []
        for c in range(N_CORES):
            res.append({
                name: np.asarray(out_arrs[i]).reshape(
                    (N_CORES,) + self.out_avals[i].shape)[c]
                for i, name in enumerate(self.out_names)})
        return res


def _get_runner(key, steps, wrap, w1, b1, w2, b2, dt16):
    if key not in _KERNEL_CACHE:
        nc = _build_nc(steps, wrap, w1, b1, w2, b2, dt16=dt16)
        _KERNEL_CACHE[key] = _Runner(nc)
    return _KERNEL_CACHE[key]


def _prep(x, w1, b1, w2, b2, steps, n, dt16=True):
    x = np.asarray(x)
    w1 = np.asarray(w1, dtype=np.float32)
    b1 = np.asarray(b1, dtype=np.float32)
    w2 = np.asarray(w2, dtype=np.float32)
    b2 = np.asarray(b2, dtype=np.float32)
    steps = int(steps)
    n = int(n)
    wrap = (n == W)
    xf = np.ascontiguousarray(x.reshape(B_FULL, H, W).astype(np.float32))
    mains, edges, comb = _build_consts(w1, w2)
    key = (steps, wrap, dt16, w1.tobytes(), b1.tobytes(), w2.tobytes(),
           b2.tobytes())
    runner = _get_runner(key, steps, wrap, w1, b1, w2, b2, dt16)
    in_maps = [{"xs": xf[c * IMGS:(c + 1) * IMGS], "mains": mains,
                "edges": edges, "comb": comb}
               for c in range(N_CORES)]
    return runner, in_maps


def kernel(x, w1, b1, w2, b2, steps, n):
    in_dtype = np.asarray(x).dtype
    runner, in_maps = _prep(x, w1, b1, w2, b2, steps, n)
    res = runner.run(in_maps)
    full = np.concatenate([r["out"] for r in res], axis=0)
    full = full.reshape(B_FULL, 1, H, W)
    return full.astype(in_dtype, copy=False)
